# revision 1
# baseline (speedup 1.0000x reference)
"""Balanced-span variable-split all-to-all (MoE dispatch) for 8 trn2 cores.
(Earlier rank-owned design preserved in kernel_v1_backup.py.)

The global valid output space (all ranks' received rows, concatenated in
(rank, row) order) is cut into 8 equal-row pieces; core k produces piece k
into its own [PIECE_MAX, H] buffer at piece-local offsets that preserve the
final (rank, row) order. Fragments are chunk-within-piece intersections --
still contiguous src/dst row ranges -- binary-decomposed into power-of-2
copies. The slot-size pool is derived from the actual splits (max demand
per size over pieces), so the SPMD program has near-zero skip slots; the
compiled program is cached per pool signature. Host unshard copies each
piece's few rank-spans into the final [W, M, H] zeros buffer.
"""
import os
import sys
import types

import numpy as np

W, M, H = 8, 16384, 1024
SENTINEL = 400_000  # OOB row for skipped slots (row units; fits int32 in bytes)
BATCH = 8

_cache = {}


def _install_profshim():
    if "antenv.axon_hooks" in sys.modules:
        return
    try:
        from trn_agent_boot.trn_boot import _ntff_profile_via_ctypes
        hook = _ntff_profile_via_ctypes("/opt/axon/libaxon_pjrt.so")
    except Exception:
        hook = None
    mod = types.ModuleType("antenv.axon_hooks")
    mod.get_axon_ntff_profile_hook = lambda: hook
    mod.set_axon_ntff_profile_hook = lambda h: None
    sys.modules["antenv.axon_hooks"] = mod


def _plan_pieces(splits):
    """Cut the concatenated valid space into 8 pieces; return per-piece
    fragment lists [(src_row, dst_local_row, n)], piece lengths, and the
    per-piece final-output span map [(r, row_start, row_end, local_start)]."""
    sp = splits.astype(np.int64)
    in_off = sp.cumsum(1) - sp          # [s, r]
    recv = sp.T                          # [r, s]
    out_off = recv.cumsum(1) - recv      # [r, s]
    totals = recv.sum(1)                 # [r]
    tot_prefix = np.concatenate([[0], totals.cumsum()])
    G = int(tot_prefix[-1])

    cuts = [round(k * G / W) for k in range(W + 1)]

    # global chunk list in (r, s) order with global start positions
    chunks = []  # (g_start, n, src_row, r, out_row_start)
    for r in range(W):
        for s in range(W):
            n = int(sp[s, r])
            if n == 0:
                continue
            g = int(tot_prefix[r] + out_off[r, s])
            chunks.append((g, n, s * M + int(in_off[s, r]), r, int(out_off[r, s])))

    frags = [[] for _ in range(W)]
    spans = [[] for _ in range(W)]
    for k in range(W):
        a, b = cuts[k], cuts[k + 1]
        if a == b:
            continue
        for g, n, src, r, orow in chunks:
            lo, hi = max(g, a), min(g + n, b)
            if lo >= hi:
                continue
            frags[k].append((src + (lo - g), lo - a, hi - lo))
        # final-output spans covered by this piece
        for r in range(W):
            ra, rb = int(tot_prefix[r]), int(tot_prefix[r + 1])
            lo, hi = max(ra, a), min(rb, b)
            if lo >= hi:
                continue
            spans[k].append((r, lo - ra, hi - ra, lo - a))
    lens = [cuts[k + 1] - cuts[k] for k in range(W)]
    return frags, lens, spans


def _decompose(frags):
    """Binary-decompose fragments -> per-piece lists of (size, src, dst)."""
    out = []
    for fl in frags:
        micro = []
        for src, dst, n in fl:
            bit = 1 << (max(n, 1).bit_length() - 1)
            while n:
                if n >= bit:
                    micro.append((bit, src, dst))
                    src += bit
                    dst += bit
                    n -= bit
                bit >>= 1
        out.append(micro)
    return out


def _pool_from_micro(micro):
    """Slot pool: per size, max count over pieces; returns descending
    [(size, count)] and per-engine slot lists."""
    demand = {}
    for ml in micro:
        c = {}
        for sz, _, _ in ml:
            c[sz] = c.get(sz, 0) + 1
        for sz, n in c.items():
            demand[sz] = max(demand.get(sz, 0), n)
    pool = sorted(demand.items(), key=lambda kv: -kv[0])
    if not pool:
        pool = [(1, 2)]  # degenerate all-empty input: two dummy skip slots
    # flat descending slot list, alternate between engines
    slots = []
    for sz, n in pool:
        slots += [sz] * n
    eng_slots = [slots[0::2], slots[1::2]]
    # Issue order per engine: big slots first (builds queue backlog), then
    # the small slots (issued while backlog is non-empty), and one large
    # anchor slot last so the queue ends with a wide 16-engine transfer
    # instead of a serial dribble of small descriptors.
    ordered = []
    for sl in eng_slots:
        big = [s for s in sl if s >= 64]
        small = [s for s in sl if s < 64]
        anchor = []
        for asz in (1024, 512, 2048, 256):
            if asz in big:
                big.remove(asz)
                anchor = [asz]
                break
        ordered.append(big + small + anchor)
    return ordered


def _build_kernel(eng_slots):
    import concourse.bacc as bacc
    import concourse.mybir as mybir
    from concourse.bass import ds

    F32 = mybir.dt.float32
    I32 = mybir.dt.int32

    nc = bacc.Bacc("TRN2", target_bir_lowering=False, debug=False, num_devices=W)
    inp = nc.dram_tensor("inp", [W * M, H], F32, kind="ExternalInput")
    ntot = sum(len(s) for s in eng_slots)
    table = nc.dram_tensor("table", [1, 2 * max(ntot, 1)], I32, kind="ExternalInput")
    out = nc.dram_tensor("out", [M, H], F32, kind="ExternalOutput")

    engines = [(nc.sync, mybir.EngineType.SP), (nc.scalar, mybir.EngineType.Activation)]
    tab_base = 0
    for (eng, eng_t), slots in zip(engines, eng_slots):
        if not slots:
            continue
        tab_sb = nc.alloc_sbuf_tensor(f"tab_{eng_t.value}", [1, 2 * len(slots)], I32)
        sem_t = nc.alloc_semaphore(f"sem_t_{eng_t.value}")
        sem_d = nc.alloc_semaphore(f"sem_d_{eng_t.value}")
        eng.sem_clear(sem_t)
        eng.sem_clear(sem_d)
        eng.dma_start(out=tab_sb[0:1, :],
                      in_=table[0:1, tab_base:tab_base + 2 * len(slots)]
                      ).then_inc(sem_t, 16)
        eng.wait_ge(sem_t, 16)
        for b0 in range(0, len(slots), BATCH):
            bsl = slots[b0:b0 + BATCH]
            _, vals = nc.values_load_multi_w_load_instructions(
                tab_sb[0:1, 2 * b0:2 * (b0 + len(bsl))],
                engines=[eng_t], skip_runtime_bounds_check=True)
            for j, sz in enumerate(bsl):
                src, dst = vals[2 * j], vals[2 * j + 1]
                eng.dma_start(out=out[ds(dst, sz), :],
                              in_=inp[ds(src, sz), :],
                              bounds_check="skip_entire_dma").then_inc(sem_d, 16)
        eng.wait_ge(sem_d, 16 * len(slots))
        tab_base += 2 * len(slots)
    nc.compile()
    return nc


def _make_tables(micro, eng_slots):
    """Pack per-piece micro-copies into the engine slot lists; skip slots
    get the OOB sentinel. Returns per-core [1, 2*ntot] int32 tables."""
    ntot = sum(len(s) for s in eng_slots)
    tables = []
    for ml in micro:
        by_size = {}
        for sz, src, dst in ml:
            by_size.setdefault(sz, []).append((src, dst))
        entries = []
        for slots in eng_slots:
            for sz in slots:
                lst = by_size.get(sz)
                if lst:
                    entries.append(lst.pop())
                else:
                    entries.append((SENTINEL, SENTINEL))
        leftover = sum(len(v) for v in by_size.values())
        assert leftover == 0, f"slot pool under-provisioned: {leftover} left"
        tables.append(np.array(entries, dtype=np.int32).reshape(1, -1))
    return tables


last_exec_time_ns = None


def kernel(input, splits, num_sm=None, **_unused):
    global last_exec_time_ns
    _install_profshim()
    from concourse.bass_utils import run_bass_kernel_spmd

    input = np.asarray(input, dtype=np.float32)
    splits = np.asarray(splits, dtype=np.int32)
    assert input.shape == (W, M, H), input.shape
    assert splits.shape == (W, W), splits.shape

    frags, lens, spans = _plan_pieces(splits)
    micro = _decompose(frags)
    eng_slots = _pool_from_micro(micro)
    key = (tuple(eng_slots[0]), tuple(eng_slots[1]))
    if key not in _cache:
        _cache[key] = _build_kernel(eng_slots)
    nc = _cache[key]

    flat = np.ascontiguousarray(input.reshape(W * M, H))
    tables = _make_tables(micro, eng_slots)
    in_maps = [{"inp": flat, "table": tables[k]} for k in range(W)]

    trace = bool(int(os.environ.get("A2A_PROFILE", "0")))
    res = run_bass_kernel_spmd(
        nc, in_maps, core_ids=list(range(W)),
        trace=trace, trace_cores=list(range(W)) if trace else None,
    )
    last_exec_time_ns = res.exec_time_ns

    out = np.zeros((W, M, H), dtype=np.float32)
    for k in range(W):
        buf = res.results[k]["out"]
        for r, ra, rb, la in spans[k]:
            out[r, ra:rb] = buf[la:la + (rb - ra)]
    return out

